# revision 1
# baseline (speedup 1.0000x reference)
"""BERT-CRF loss kernel for Trainium2 (8 NeuronCores, data-parallel over sentences).

Math: loss = sum_b(forward_b - cumsum(gold)_b) for a CRF whose forward scan runs
over the flattened B*S steps (batch carryover).  The log-semiring scan is
reassociated into per-chunk (L=16 positions) transfer matrices computed on
device in scaled probability space:

  feats[pos,t]   = hidden @ W.T + b          (PE, pos-major so the per-position
                                              max over live tags is a free-dim
                                              reduce and the exp bias is
                                              per-partition)
  EF = exp(feats - m)                         (ACT)
  chunk scan: A <- diag(EF_s) @ (E_live @ A)  (PE matmul + DVE broadcast-mul,
                                              bf16, rows = 10 live tags)

Host combines the 2048 tiny [10,12] chunk matrices sequentially in f64
(log-semiring matvec), reads off sentence-end vectors, and computes the gold
score from the shipped feats.  START/STOP rows are structurally zero in the
scan; their contributions are exactly 0 at float precision (e^-10000).

Per core: 8 sentences = 4096 positions; hidden arrives pre-transposed
[768, 4096] so the h-contraction sits on partitions.
"""
import numpy as np
import ml_dtypes
from contextlib import ExitStack

import concourse.bass as bass
import concourse.mybir as mybir
from concourse.tile import TileContext
from concourse.tile_rust import add_dep_helper
from concourse.bass_utils import run_bass_kernel_spmd

B, S, H, T = 64, 512, 768, 12
START, STOP, NEG = 10, 11, -10000.0
L = 16                   # chunk length (positions per transfer matrix)
NCORES = 8
P_CORE = B * S // NCORES  # 4096 positions per core
NCH = P_CORE // L         # 256 chunks per core
BF16 = ml_dtypes.bfloat16

F32 = mybir.dt.float32
BF = mybir.dt.bfloat16


def _build_nc():
    nc = bass.Bass()
    hiddenT = nc.declare_dram_parameter("hiddenT", [H, P_CORE], F32, isOutput=False)
    wt = nc.declare_dram_parameter("wt", [H, T], F32, isOutput=False)
    bvec = nc.declare_dram_parameter("bvec", [1, T], F32, isOutput=False)
    ones1 = nc.declare_dram_parameter("ones1", [1, 128], F32, isOutput=False)
    ident = nc.declare_dram_parameter("ident", [128, 128], BF, isOutput=False)
    etl = nc.declare_dram_parameter("etl", [128, 32], BF, isOutput=False)
    e40 = nc.declare_dram_parameter("e40", [96, 384], BF, isOutput=False)
    zeros = nc.declare_dram_parameter("zeros", [96, 512], BF, isOutput=False)
    feats_pm = nc.declare_dram_parameter("feats_pm", [P_CORE, T], F32, isOutput=True)
    m_out = nc.declare_dram_parameter("m_out", [128, 32], F32, isOutput=True)
    a_out = nc.declare_dram_parameter("a_out", [288, 384], BF, isOutput=True)

    SG_OF = [0, 0, 0, 1, 1, 1, 2, 2]
    SLOT_OF = [0, 1, 2, 0, 1, 2, 0, 1]
    last_insts = {}
    out_dmas = []

    with ExitStack() as ctx:
        tc = ctx.enter_context(TileContext(nc))
        const_pool = ctx.enter_context(tc.tile_pool(name="const", bufs=1))
        hid_pool = ctx.enter_context(tc.tile_pool(name="hid", bufs=48))
        mneg_pool = ctx.enter_context(tc.tile_pool(name="mneg", bufs=32))
        ef_pool = ctx.enter_context(tc.tile_pool(name="efp", bufs=32))
        a_pool = ctx.enter_context(tc.tile_pool(name="apool", bufs=48))
        psf_pool = ctx.enter_context(tc.tile_pool(name="psf", bufs=2, space="PSUM"))
        pst_pool = ctx.enter_context(tc.tile_pool(name="pst", bufs=2, space="PSUM"))
        pss_pool = ctx.enter_context(tc.tile_pool(name="pss", bufs=3, space="PSUM"))
        warm_pool = ctx.enter_context(tc.tile_pool(name="warm", bufs=1, space="PSUM"))

        # ---- constants (each DMA has no deps -> 0 waits) ----
        wt_sb = const_pool.tile([128, 6 * T], F32)
        nc.gpsimd.dma_start(
            out=wt_sb[:, :].rearrange("p (k t) -> p k t", t=T),
            in_=wt[:, :].rearrange("(k p) t -> p k t", p=128),
        )
        b_sb = const_pool.tile([1, T], F32)
        nc.gpsimd.dma_start(out=b_sb[:, :], in_=bvec[:, :])
        ones_sb = const_pool.tile([1, 128], F32)
        nc.gpsimd.dma_start(out=ones_sb[:, :], in_=ones1[:, :])
        ident_sb = const_pool.tile([128, 128], BF)
        nc.gpsimd.dma_start(out=ident_sb[:, :], in_=ident[:, :])
        etl_sb = const_pool.tile([128, 32], BF)
        nc.gpsimd.dma_start(out=etl_sb[:, :], in_=etl[:, :])
        e40_sb = const_pool.tile([96, 384], BF)
        nc.gpsimd.dma_start(out=e40_sb[:, :], in_=e40[:, :])
        m_all = const_pool.tile([128, 32], F32)
        feats_big = const_pool.tile([128, 384], F32)
        ef40_tiles = [const_pool.tile([96, 512], BF, name=f"ef40_{i}",
                                      tag=f"ef40_{i}") for i in range(3)]
        for i in range(3):
            nc.gpsimd.dma_start(out=ef40_tiles[i][:, :], in_=zeros[:, :])

        # ---- warm-up touches: after these, no instruction needs more than
        # one semaphore wait (ISA sync-slot limit on LDW / DMA descriptors).
        wp = warm_pool.tile([128, 128], F32)
        nc.tensor.matmul(wp[0:12, 0:12], lhsT=wt_sb[:, 0:T], rhs=wt_sb[:, 0:T],
                         start=True, stop=True)
        nc.tensor.matmul(wp[0:128, 0:12], lhsT=ones_sb[:, :],
                         rhs=ones_sb[0:1, 0:T], start=True, stop=True)
        nc.tensor.matmul(wp[0:12, 0:12], lhsT=b_sb[:, :], rhs=b_sb[:, :],
                         start=True, stop=True)
        nc.tensor.matmul(wp[0:32, 0:32], lhsT=etl_sb[0:10, :],
                         rhs=etl_sb[0:10, :], start=True, stop=True)
        nc.tensor.matmul(wp[0:128, 0:1], lhsT=ident_sb[:, :],
                         rhs=ident_sb[:, 0:1], start=True, stop=True)
        scr_v = const_pool.tile([1, 8], BF)
        nc.vector.tensor_copy(scr_v[0:1, 0:1], e40_sb[0:1, 0:1])
        scr_a = const_pool.tile([1, 8], F32)
        for i in range(3):
            nc.vector.tensor_copy(scr_v[0:1, 4 + i:5 + i],
                                  ef40_tiles[i][0:1, 0:1])
            nc.scalar.activation(scr_a[0:1, i:i + 1], ef40_tiles[i][0:1, 0:1],
                                 mybir.ActivationFunctionType.Copy)

        # ---- input stream: 48 distinct tiles, no reuse -> 0-wait DMAs ----
        hid_sb = {}
        in_dmas = []
        for g in range(8):
            for hs in range(6):
                t = hid_pool.tile([128, 512], F32, name=f"hid_{g}_{hs}", tag="hid")
                di = nc.gpsimd.dma_start(
                    out=t[:, :],
                    in_=hiddenT[hs * 128:(hs + 1) * 128, g * 512:(g + 1) * 512],
                )
                in_dmas.append(di)
                hid_sb[(g, hs)] = t

        def feats_block(g):
            sg, slot = SG_OF[g], SLOT_OF[g]
            nc.tensor.matmul(wp[0:1, 0:1], lhsT=hid_sb[(g, 0)][:, 0:1],
                             rhs=hid_sb[(g, 0)][:, 0:1], start=True, stop=True)
            for pt in range(4):
                col = g * 4 + pt
                psf = psf_pool.tile([128, T], F32)
                for hs in range(6):
                    nc.tensor.matmul(
                        psf[:, :],
                        lhsT=hid_sb[(g, hs)][:, pt * 128:(pt + 1) * 128],
                        rhs=wt_sb[:, hs * T:(hs + 1) * T],
                        start=(hs == 0), stop=False,
                    )
                nc.tensor.matmul(
                    psf[:, :], lhsT=ones_sb[:, :], rhs=b_sb[:, :],
                    start=False, stop=True,
                )
                nc.scalar.activation(
                    feats_big[:, col * T:(col + 1) * T], psf[:, :],
                    mybir.ActivationFunctionType.Copy)
                nc.vector.reduce_max(
                    out=m_all[:, col:col + 1],
                    in_=feats_big[:, col * T:col * T + 10],
                    axis=mybir.AxisListType.X,
                )
                mneg = mneg_pool.tile([128, 1], F32, name=f"mneg_{col}", tag="mneg")
                nc.scalar.activation(
                    mneg[:, :], m_all[:, col:col + 1],
                    mybir.ActivationFunctionType.Copy, scale=-1.0,
                )
                ef_pos = ef_pool.tile([128, T], BF, name=f"efpos_{col}", tag="efpos")
                nc.scalar.activation(
                    ef_pos[:, :], psf[:, :], mybir.ActivationFunctionType.Exp,
                    bias=mneg[:, 0:1], scale=1.0,
                )
                pst = pst_pool.tile([T, 128], BF)
                nc.tensor.transpose(pst[:, :], ef_pos[:, :], ident_sb[:, :])
                ai = nc.scalar.activation(
                    ef40_tiles[sg][slot * 32:slot * 32 + 10,
                                   pt * 128:(pt + 1) * 128],
                    pst[0:10, :], mybir.ActivationFunctionType.Copy,
                )
                last_insts["act"] = ai

        def scan_sg(sg):
            nslots = 3
            ef40 = ef40_tiles[sg]
            # absorb the ACT (EF writes) wait into the DVE clock up front
            nc.vector.tensor_copy(scr_v[0:1, 1 + sg:2 + sg], ef40[0:1, 0:1])
            At = None
            for s in range(16):
                ef_base = ef40[:, s::16]            # [128, 32] (chunk stride L)
                ef_ap = bass.AP(ef_base.tensor, ef_base.offset,
                                list(ef_base.ap) + [[0, T]])  # [128, 32, 12]
                At2 = a_pool.tile([96, 384], BF, name=f"at_{sg}_{s}", tag="at")
                if s == 0:
                    nc.vector.tensor_mul(
                        At2[:, :].rearrange("p (c j) -> p c j", j=T),
                        e40_sb[:, :].rearrange("p (c j) -> p c j", j=T),
                        ef_ap,
                    )
                else:
                    # absorber 1: pull the DVE (At ready) tick into PE clock
                    ab1 = nc.tensor.matmul(wp[0:1, 0:1], lhsT=At[0:1, 0:1],
                                           rhs=At[0:1, 0:1], start=True,
                                           stop=True)
                    ps = pss_pool.tile([96, 384], F32)
                    # absorber 2: dummy first-writer carries the PSUM
                    # bank-reuse hazard wait
                    ab2 = nc.tensor.matmul(ps[0:1, 0:1], lhsT=etl_sb[0:1, 0:1],
                                           rhs=etl_sb[0:1, 0:1],
                                           start=True, stop=True,
                                           skip_group_check=True)
                    add_dep_helper(ab2.ins, ab1.ins, False,
                                   "absorber ordering")
                    for u in range(nslots):
                        mi = nc.tensor.matmul(
                            ps[u * 32:(u + 1) * 32, :],
                            lhsT=etl_sb[u * 32:u * 32 + 10, :],
                            rhs=At[u * 32:u * 32 + 10, :],
                            start=True, stop=True,
                            skip_group_check=True,
                        )
                        last_insts["pe"] = mi
                    vi = nc.vector.tensor_mul(
                        At2[:, :].rearrange("p (c j) -> p c j", j=T),
                        ps[:, :].rearrange("p (c j) -> p c j", j=T),
                        ef_ap,
                    )
                    last_insts["dve"] = vi
                At = At2
            oi = nc.sync.dma_start(out=a_out[sg * 96:(sg + 1) * 96, :],
                                    in_=At[:, :])
            out_dmas.append(oi)

        for g in range(3):
            feats_block(g)
        scan_sg(0)
        for g in range(3, 6):
            feats_block(g)
        scan_sg(1)
        for g in range(6, 8):
            feats_block(g)
        scan_sg(2)
        oi = nc.sync.dma_start(
            out=feats_pm[:, :].rearrange("(c p) t -> p c t", p=128),
            in_=feats_big[:, :].rearrange("p (c t) -> p c t", t=T),
        )
        out_dmas.append(oi)
        oi = nc.sync.dma_start(out=m_out[:, :], in_=m_all[:, :])
        out_dmas.append(oi)
        # Pre-absorb every proc's clock into SP one dep at a time, so the
        # Tile tail drain does not need a multi-sem wait.
        for dep in in_dmas[-8:] + out_dmas + list(last_insts.values()):
            nop = nc.sync.nop()
            add_dep_helper(nop.ins, dep.ins, True, "drain preclear")
    return nc


_NC_CACHE = None


def _get_nc():
    global _NC_CACHE
    if _NC_CACHE is None:
        _NC_CACHE = _build_nc()
    return _NC_CACHE


def _build_etl128(E):
    e = np.zeros((128, 32), np.float32)
    for slot in range(3):
        e[slot * 32:slot * 32 + 10, 0:10] = E[:10, :10].T
    return e.astype(BF16)


def _build_e128(E):
    e = np.zeros((96, 384), np.float32)
    for slot in range(3):
        e[slot * 32:slot * 32 + 10, :] = np.tile(E[:10, :], (1, 32))
    return e.astype(BF16)


def _run_device(hidden, W, b, transitions, trace=False, tmpdir=None):
    E = np.exp(transitions.astype(np.float64))
    E[START, :] = 0.0            # structurally dead (no transition out of a
    E[STOP, :] = 0.0             # state that can't receive / into one that
    E[:, STOP] = 0.0             # can't send): contributions are e^-10000 = 0.
    E = E.astype(np.float32)

    wt_np = np.ascontiguousarray(W.T).astype(np.float32)
    in_common = {
        "wt": wt_np,
        "bvec": b.reshape(1, T).astype(np.float32),
        "ones1": np.ones((1, 128), np.float32),
        "ident": np.eye(128).astype(BF16),
        "etl": _build_etl128(E),
        "e40": _build_e128(E),
        "zeros": np.zeros((96, 512), BF16),
    }
    flat = hidden.reshape(B * S, H)
    in_maps = []
    for c in range(NCORES):
        hT = np.ascontiguousarray(
            flat[c * P_CORE:(c + 1) * P_CORE].T).astype(np.float32)
        d = dict(in_common)
        d["hiddenT"] = hT
        in_maps.append(d)

    res = run_bass_kernel_spmd(
        _get_nc(), in_maps, list(range(NCORES)), trace=trace, tmpdir=tmpdir)
    return res


def _host_combine(results, transitions, tags):
    feats = np.concatenate(
        [np.asarray(r["feats_pm"]) for r in results], axis=0)  # [B*S, T] f32
    m_flat = np.concatenate(
        [np.asarray(r["m_out"]).T.reshape(P_CORE) for r in results])  # [B*S]
    # chunk matrices [2048, 10, 12]
    A = np.zeros((NCORES * NCH, 10, T), np.float32)
    for c, r in enumerate(results):
        a = np.asarray(r["a_out"]).astype(np.float32)  # [288, 384]
        SG_OF = [0, 0, 0, 1, 1, 1, 2, 2]
        SLOT_OF = [0, 1, 2, 0, 1, 2, 0, 1]
        for g in range(8):
            sg, slot = SG_OF[g], SLOT_OF[g]
            blk = a[sg * 96 + slot * 32: sg * 96 + slot * 32 + 10, :]  # [10,384]
            A[c * NCH + g * 32:c * NCH + (g + 1) * 32] = (
                blk.reshape(10, 32, T).transpose(1, 0, 2))
    n_chunks = NCORES * NCH
    scale = m_flat.astype(np.float64).reshape(n_chunks, L).sum(axis=1)
    with np.errstate(divide="ignore"):
        logP = np.log(A.astype(np.float64)) + scale[:, None, None]

    v = np.full(T, NEG, np.float64)
    v[START] = 0.0
    last = np.zeros((B, T), np.float64)
    cps = S // L
    err = np.errstate(invalid="ignore", divide="ignore", over="ignore")
    err.__enter__()
    for c in range(n_chunks):
        x = logP[c] + v[None, :]
        mx = np.max(x, axis=1)
        mx_safe = np.where(np.isfinite(mx), mx, 0.0)
        with np.errstate(invalid="ignore"):
            vl = mx + np.log(np.sum(np.exp(x - mx_safe[:, None]), axis=1))
        vl = np.where(np.isfinite(mx), vl, -np.inf)
        v = np.concatenate([vl, [-np.inf, -np.inf]])
        if (c + 1) % cps == 0:
            last[(c + 1) // cps - 1] = v
    x = last + transitions[STOP][None, :].astype(np.float64)
    mx = x.max(axis=1)
    forward_score = mx + np.log(np.exp(x - mx[:, None]).sum(axis=1))  # [B]
    err.__exit__(None, None, None)

    tags_ext = np.concatenate(
        [np.full((B, 1), START, dtype=tags.dtype), tags], axis=1)
    prev, nxt = tags_ext[:, :-1], tags_ext[:, 1:]
    trans_sc = transitions[nxt, prev].astype(np.float64).sum(axis=1)
    featsb = feats.reshape(B, S, T)
    emit_sc = np.take_along_axis(
        featsb.astype(np.float64), nxt[..., None].astype(np.int64), axis=2
    )[..., 0].sum(axis=1)
    gold = trans_sc + emit_sc + transitions[STOP, tags_ext[:, -1]].astype(np.float64)
    gold_cum = np.cumsum(gold)
    out = np.sum(forward_score - gold_cum)
    return np.array([out], dtype=np.float32)


def kernel(hidden, W, b, transitions, tags, _trace=False, _tmpdir=None):
    hidden = np.asarray(hidden, dtype=np.float32)
    W = np.asarray(W, dtype=np.float32)
    b = np.asarray(b, dtype=np.float32)
    transitions = np.asarray(transitions, dtype=np.float32)
    tags = np.asarray(tags)
    res = _run_device(hidden, W, b, transitions, trace=_trace, tmpdir=_tmpdir)
    out = _host_combine(res.results, transitions, tags)
    if _trace:
        return out, res
    return out



# revision 7
# speedup vs baseline: 4.5350x; 4.5350x over previous
"""BERT-CRF loss kernel for Trainium2 (8 NeuronCores, data-parallel over positions).

Math: loss = sum_b(forward_b - cumsum(gold)_b) for a CRF whose forward scan runs
over the flattened B*S steps (batch carryover).  The log-semiring scan is
reassociated into per-chunk (L=4 positions) transfer matrices computed on
device in scaled probability space:

  feats[pos,t] = hidden @ W.T + b     (PE, fp8, pos-major)
  m[pos]       = max over live tags   (Pool reduce)
  fsub         = feats - m            (Pool sub, f32, shipped for gold score)
  EF           = exp(fsub)            (ACT, bf16)
  chunk scan: At <- (Eblk.T @ At) * EF_s   (PE matmul + DVE j-major bcast mul)

Positions are column-permuted on the host so that each scan step reads a
contiguous 32-column EF slice, and the 8 sentences (groups) are pair-packed
at partition offsets {0,16} inside 32-aligned slots (PE transposes write
[32,128] blocks at legal partition starts).

Host combines the 8192 chunk matrices (f64, tree per sentence + sequential
sentence carry) and computes the gold score from the shipped fsub + m.
"""
import numpy as np
import ml_dtypes
from contextlib import ExitStack

import concourse.bass as bass
import concourse.mybir as mybir
from concourse.tile import TileContext
from concourse.bass_utils import run_bass_kernel_spmd

B, S, H, T = 64, 512, 768, 12
START, STOP, NEG = 10, 11, -10000.0
L = 4                      # chunk length (positions per transfer matrix)
NCORES = 8
P_CORE = B * S // NCORES   # 4096 positions per core
G = 8                      # sentences (groups) per core
KPG = S // L               # 128 chunks per group
NQ = 4                     # quarter chains
CPQ = KPG // NQ            # 32 chunks per group per quarter
NLIVE = 10

FP8NP = ml_dtypes.float8_e4m3fn
BF16 = ml_dtypes.bfloat16

F32 = mybir.dt.float32
BF = mybir.dt.bfloat16
FP8 = mybir.dt.float8e4

# device column <-> original position permutation (per core)
# position q = g*512 + k*L + s ; Q = k//CPQ ; c = k%CPQ
# col = Q*1024 + g*128 + s*32 + c
_cols = np.arange(P_CORE)
_Q = _cols // 1024
_g = (_cols % 1024) // 128
_s = (_cols % 128) // 32
_c = _cols % 32
PERM = (_g * S + (_Q * CPQ + _c) * L + _s)   # PERM[col] = original position


def _build_nc():
    nc = bass.Bass()
    hidT = nc.declare_dram_parameter("hidT", [H, P_CORE], FP8, isOutput=False)
    cf8 = nc.declare_dram_parameter("cf8", [128, 212], FP8, isOutput=False)
    cb16 = nc.declare_dram_parameter("cb16", [128, 576], BF, isOutput=False)
    fm_out = nc.declare_dram_parameter("fm_out", [128, 416], F32, isOutput=True)
    a_out = nc.declare_dram_parameter("a_out", [128, NQ * 320], BF, isOutput=True)

    with ExitStack() as ctx:
        tc = ctx.enter_context(TileContext(nc))
        const_pool = ctx.enter_context(tc.tile_pool(name="const", bufs=1))
        hid_pool = ctx.enter_context(tc.tile_pool(name="hid", bufs=12))
        sb_pool = ctx.enter_context(tc.tile_pool(name="sb", bufs=1))
        at_pool = ctx.enter_context(tc.tile_pool(name="at", bufs=8))
        fp_pool = ctx.enter_context(tc.tile_pool(name="fps", bufs=1, space="PSUM"))
        eft_pool = ctx.enter_context(tc.tile_pool(name="eftp", bufs=1, space="PSUM"))
        ps_pool = ctx.enter_context(tc.tile_pool(name="pss", bufs=4, space="PSUM"))
        warm_pool = ctx.enter_context(tc.tile_pool(name="warm", bufs=1, space="PSUM"))

        # ---- persistent SBUF tiles ----
        cf8_sb = const_pool.tile([128, 212], FP8)
        cb16_sb = const_pool.tile([128, 576], BF)
        efpos = sb_pool.tile([128, 512], BF)       # pos-major EF, pair-packed
        ef_sb = sb_pool.tile([128, 512], BF)       # tag-major EF (scan operand)
        fm_sb = sb_pool.tile([128, 416], F32)      # m (cols 0:32) | fsub (32:416)
        a_sb = sb_pool.tile([128, NQ * 320], BF)   # final chunk matrices

        fp_ps = fp_pool.tile([128, 384], F32)      # feats psum (32 blocks x 12)
        eft_ps = eft_pool.tile([128, 512], BF)     # transposed EF psum

        # ---- pad init: zero the 4-wide pad columns of efpos (cols g*16+12..16)
        nc.gpsimd.memset(
            bass.AP(efpos.tensor, efpos[:, 12:16].offset,
                    [efpos[:, :].ap[0], [16, 32], [1, 4]]),
            0.0,
        )

        # ---- input DMAs, split across the three DMA-capable queues ----
        # half h covers cols h*2048:(h+1)*2048 ; row block hs covers 128 h-rows
        hid = {}
        def hdma(eng, hs, h):
            t = hid_pool.tile([128, 2048], FP8, name=f"hid_{hs}_{h}", tag="hid")
            eng.dma_start(out=t[:, :],
                          in_=hidT[hs * 128:(hs + 1) * 128, h * 2048:(h + 1) * 2048])
            hid[(hs, h)] = t

        # Pool: 4 hid ; SP: 4 hid + cb16 ; ACT: cf8 + 4 hid
        hdma(nc.gpsimd, 0, 0)
        hdma(nc.gpsimd, 1, 0)
        nc.sync.dma_start(out=cb16_sb[:, :], in_=cb16[:, :])
        hdma(nc.sync, 2, 0)
        hdma(nc.sync, 3, 0)
        nc.scalar.dma_start(out=cf8_sb[:, :], in_=cf8[:, :])
        hdma(nc.scalar, 4, 0)
        hdma(nc.scalar, 5, 0)
        hdma(nc.gpsimd, 0, 1)
        hdma(nc.gpsimd, 1, 1)
        hdma(nc.sync, 2, 1)
        hdma(nc.sync, 3, 1)
        hdma(nc.scalar, 4, 1)
        hdma(nc.scalar, 5, 1)

        ident = cb16_sb[:, 0:128]
        eblk = cb16_sb[:, 128:256]
        e40jm = cb16_sb[:, 256:576]

        # ---- warmups: ramp PE p-state, load ACT exp table early ----
        wp = warm_pool.tile([128, 384], F32)
        scr = const_pool.tile([1, 16], BF)
        nc.scalar.activation(scr[0:1, 0:8], cf8_sb[0:1, 0:8],
                             mybir.ActivationFunctionType.Exp)
        for _ in range(3):
            nc.tensor.matmul(wp[:, 0:212], lhsT=cf8_sb[:, 0:128],
                             rhs=cf8_sb[:, 0:212], start=True, stop=True)

        def feats_quarter(Q):
            h = Q // 2
            for g in range(G):
                blk = fp_ps[:, (Q * 8 + g) * 12:(Q * 8 + g) * 12 + 12]
                for hs in range(6):
                    nc.tensor.matmul(
                        blk,
                        lhsT=hid[(hs, h)][:, (Q % 2) * 1024 + g * 128:
                                          (Q % 2) * 1024 + (g + 1) * 128],
                        rhs=cf8_sb[:, hs * 12:(hs + 1) * 12],
                        start=(hs == 0), stop=False,
                        skip_group_check=True,
                    )
                nc.tensor.matmul(
                    blk, lhsT=cf8_sb[0:1, 72:200], rhs=cf8_sb[0:1, 200:212],
                    start=False, stop=True, skip_group_check=True,
                )
            fq = fp_ps[:, Q * 96:(Q + 1) * 96]
            fq3 = fq.rearrange("p (b j) -> p b j", j=12)
            # m = max over live tags (DVE)
            nc.vector.reduce_max(
                out=fm_sb[:, Q * 8:(Q + 1) * 8],
                in_=bass.AP(fq3.tensor, fq3.offset,
                            [fq3.ap[0], fq3.ap[1], [1, NLIVE]]),
                axis=mybir.AxisListType.X,
            )
            # fsub = feats - m (Pool)
            msl = fm_sb[:, Q * 8:(Q + 1) * 8]
            m_b = bass.AP(msl.tensor, msl.offset,
                          [msl.ap[0], msl.ap[1], [0, 12]])
            nc.gpsimd.tensor_sub(
                fm_sb[:, 32 + Q * 96:32 + (Q + 1) * 96]
                .rearrange("p (b j) -> p b j", j=12),
                fq3, m_b,
            )
            # EF = exp(fsub) into pair-packed layout (ACT)
            eo = efpos[:, Q * 128:(Q + 1) * 128]
            eo3 = bass.AP(eo.tensor, eo.offset, [eo.ap[0], [16, 8], [1, 12]])
            nc.scalar.activation(
                eo3,
                fm_sb[:, 32 + Q * 96:32 + (Q + 1) * 96]
                .rearrange("p (b j) -> p b j", j=12),
                mybir.ActivationFunctionType.Exp,
            )
            # transpose pairs into tag-major psum
            for v in range(4):
                nc.tensor.transpose(
                    eft_ps[32 * v:32 * v + 32, Q * 128:(Q + 1) * 128],
                    efpos[:, Q * 128 + v * 32:Q * 128 + v * 32 + 32],
                    ident,
                    tile_position=(0, 32 * v),
                )
            # copy psum -> sbuf (DVE, 2x)
            nc.vector.tensor_copy(ef_sb[:, Q * 128:(Q + 1) * 128],
                                  eft_ps[:, Q * 128:(Q + 1) * 128])

        def ef_ap(Q, s):
            base = ef_sb[:, Q * 128 + s * 32:Q * 128 + s * 32 + 32]
            return bass.AP(base.tensor, base.offset,
                           [base.ap[0], [0, NLIVE], base.ap[1]])

        def scan_quarter(Q):
            at = at_pool.tile([128, 320], BF, name=f"at_{Q}_0", tag="at")
            nc.vector.tensor_mul(
                at[:, :].rearrange("p (j c) -> p j c", c=32),
                e40jm.rearrange("p (j c) -> p j c", c=32),
                ef_ap(Q, 0),
            )
            for s in range(1, L):
                ps = ps_pool.tile([128, 320], F32)
                nc.tensor.matmul(ps[:, :], lhsT=eblk, rhs=at[:, :],
                                 start=True, stop=True)
                if s < L - 1:
                    at2 = at_pool.tile([128, 320], BF, name=f"at_{Q}_{s}", tag="at")
                    out_ap = at2[:, :].rearrange("p (j c) -> p j c", c=32)
                else:
                    at2 = None
                    sl = a_sb[:, Q * 320:(Q + 1) * 320]
                    out_ap = sl.rearrange("p (j c) -> p j c", c=32)
                nc.vector.tensor_mul(
                    out_ap,
                    ps[:, :].rearrange("p (j c) -> p j c", c=32),
                    ef_ap(Q, s),
                )
                at = at2
            nc.sync.dma_start(out=a_out[:, Q * 320:(Q + 1) * 320],
                              in_=a_sb[:, Q * 320:(Q + 1) * 320])

        for Q in range(NQ):
            feats_quarter(Q)
            scan_quarter(Q)
        nc.sync.dma_start(out=fm_out[:, :], in_=fm_sb[:, :])
    return nc


_NC_CACHE = None


def _get_nc():
    global _NC_CACHE
    if _NC_CACHE is None:
        _NC_CACHE = _build_nc()
    return _NC_CACHE


def _build_consts(W, b, transitions):
    E = np.exp(transitions.astype(np.float64))
    E[START, :] = 0.0
    E[STOP, :] = 0.0
    E[:, STOP] = 0.0
    E = E.astype(np.float32)

    cf8 = np.zeros((128, 212), np.float32)
    # wt: cf8[p, hs*12+t] = W[t, hs*128+p]
    cf8[:, 0:72] = W.T.reshape(6, 128, T).transpose(1, 0, 2).reshape(128, 72)
    cf8[0, 72:200] = 1.0
    cf8[0, 200:212] = b
    cf8 = cf8.astype(FP8NP)

    cb16 = np.zeros((128, 576), np.float32)
    cb16[:, 0:128] = np.eye(128)
    # eblk: eblk[32v+off+j, 32v+off+i] = E[i, j]  (live 10x10)
    for v in range(4):
        for off in (0, 16):
            o = 32 * v + off
            cb16[o:o + NLIVE, 128 + o:128 + o + NLIVE] = E[:NLIVE, :NLIVE].T
    # e40jm: e40jm[32v+off+i, j*32+c] = E[i, j]
    blk = np.zeros((32, 320), np.float32)
    for off in (0, 16):
        for i in range(NLIVE):
            for j in range(NLIVE):
                blk[off + i, j * 32:(j + 1) * 32] = E[i, j]
    for v in range(4):
        cb16[32 * v:32 * v + 32, 256:576] = blk
    cb16 = cb16.astype(BF16)
    return cf8, cb16


def _run_device(hidden, W, b, transitions, trace=False, tmpdir=None):
    cf8, cb16 = _build_consts(W, b, transitions)
    flat = hidden.reshape(B * S, H)
    in_maps = []
    for core in range(NCORES):
        blk = flat[core * P_CORE:(core + 1) * P_CORE]        # [4096, 768]
        hT = np.ascontiguousarray(blk[PERM].T).astype(FP8NP)  # [768, 4096]
        in_maps.append({"hidT": hT, "cf8": cf8, "cb16": cb16})
    return run_bass_kernel_spmd(
        _get_nc(), in_maps, list(range(NCORES)), trace=trace, tmpdir=tmpdir)


def _logsumexp(x, axis):
    mx = np.max(x, axis=axis)
    mx_safe = np.where(np.isfinite(mx), mx, 0.0)
    out = mx + np.log(np.sum(np.exp(x - np.expand_dims(mx_safe, axis)), axis=axis))
    return np.where(np.isfinite(mx), out, -np.inf)


def _host_combine(results, transitions, tags):
    trans = transitions.astype(np.float64)
    err = np.errstate(invalid="ignore", divide="ignore", over="ignore")
    err.__enter__()

    # unpack fm_out: m and fsub in device (permuted) order -> original order
    feats = np.zeros((NCORES, P_CORE, T), np.float64)
    m_all = np.zeros((NCORES, P_CORE), np.float64)
    for core, r in enumerate(results):
        fm = np.asarray(r["fm_out"]).astype(np.float64)   # [128, 416]
        # device col layout: col = Q*1024 + g*128 + s*32 + c ; tile t=Q*8+g,
        # partition p = s*32+c ; m = fm[:, t], fsub = fm[:, 32+12t+j]
        m_dev = fm[:, 0:32]            # [p, t]
        fs_dev = fm[:, 32:416].reshape(128, 32, 12)   # [p, t, j]
        # device column index for (t, p): col = (t//8)*1024 + (t%8)*128 + p
        tt = np.arange(32)
        pp = np.arange(128)
        colidx = (tt[None, :] // 8) * 1024 + (tt[None, :] % 8) * 128 + pp[:, None]
        q = PERM[colidx]               # [p, t] original position
        m_all[core, q] = m_dev
        feats[core, q, :] = fs_dev + m_dev[:, :, None]

    feats = feats.reshape(B, S, T)     # [B, S, T] (= fsub + m, exact device feats)
    m_all = m_all.reshape(B, S)

    # unpack chunk matrices: A[b, k][i, j] (live 10x10), log + scale
    logA = np.zeros((B, KPG, NLIVE, NLIVE), np.float64)
    for core, r in enumerate(results):
        a = np.asarray(r["a_out"]).astype(np.float64)     # [128, 1280]
        a4 = a.reshape(128, NQ, NLIVE, 32)                 # [p, Q, j, c]
        for g in range(G):
            rows = 32 * (g // 2) + 16 * (g % 2)
            blkm = a4[rows:rows + NLIVE]                   # [i, Q, j, c]
            logA[core * G + g] = np.log(blkm).transpose(1, 3, 0, 2).reshape(
                KPG, NLIVE, NLIVE)
    scale = m_all.reshape(B, KPG, L).sum(axis=2)           # [B, KPG]
    logA = logA + scale[:, :, None, None]

    # first global chunk: explicit recurrence from init (full 12-state)
    v0 = np.full(T, NEG, np.float64)
    v0[START] = 0.0
    for s in range(L):
        v0 = _logsumexp(trans[None, :, :] + v0[None, None, :], axis=2)[0] \
            + feats[0, s]
    # replace chunk (0,0) with identity in the tree
    ident = np.full((NLIVE, NLIVE), -np.inf)
    np.fill_diagonal(ident, 0.0)
    logA[0, 0] = ident

    # tree-combine the 128 chunk mats of each sentence -> one mat per sentence
    mats = logA.reshape(B * KPG, NLIVE, NLIVE)
    n = B * KPG
    while n > B:
        A2 = mats[0::2]        # earlier chunk
        B2 = mats[1::2]        # later chunk
        x = B2[:, :, :, None] + A2[:, None, :, :]          # [n/2, i, j, k]
        mats = _logsumexp(x, axis=2)
        n //= 2

    # sequential carry across sentences
    last = np.zeros((B, T), np.float64)
    v = v0.copy()
    for b in range(B):
        if b == 0:
            vl = _logsumexp(mats[0] + v[None, :NLIVE], axis=1)
        else:
            vl = _logsumexp(mats[b] + v[None, :NLIVE], axis=1)
        v = np.concatenate([vl, [-np.inf, -np.inf]])
        last[b] = v
        if b + 1 < B:
            # start of next sentence: nothing special (carryover CRF)
            pass
    forward_score = _logsumexp(last + trans[STOP][None, :], axis=1)
    err.__exit__(None, None, None)

    tags = np.asarray(tags)
    tags_ext = np.concatenate(
        [np.full((B, 1), START, dtype=tags.dtype), tags], axis=1)
    prev, nxt = tags_ext[:, :-1], tags_ext[:, 1:]
    trans_sc = trans[nxt, prev].sum(axis=1)
    emit_sc = np.take_along_axis(
        feats, nxt[..., None].astype(np.int64), axis=2)[..., 0].sum(axis=1)
    gold = trans_sc + emit_sc + trans[STOP, tags_ext[:, -1]]
    gold_cum = np.cumsum(gold)
    out = np.sum(forward_score - gold_cum)
    return np.array([out], dtype=np.float32)


def kernel(hidden, W, b, transitions, tags, _trace=False, _tmpdir=None):
    hidden = np.asarray(hidden, dtype=np.float32)
    W = np.asarray(W, dtype=np.float32)
    b = np.asarray(b, dtype=np.float32)
    transitions = np.asarray(transitions, dtype=np.float32)
    res = _run_device(hidden, W, b, transitions, trace=_trace, tmpdir=_tmpdir)
    out = _host_combine(res.results, transitions, tags)
    if _trace:
        return out, res
    return out


# revision 12
# speedup vs baseline: 4.6913x; 1.0345x over previous
"""BERT-CRF loss kernel for Trainium2 (8 NeuronCores, data-parallel over positions).

Math: loss = sum_b(forward_b - cumsum(gold)_b) for a CRF whose forward scan runs
over the flattened B*S steps (batch carryover).  The log-semiring scan is
reassociated into per-chunk (L=4 positions) transfer matrices computed on
device in scaled probability space:

  feats[pos,t] = hidden @ W.T + b     (PE, fp8, pos-major)
  m[pos]       = max over live tags   (Pool reduce)
  fsub         = feats - m            (Pool sub, f32, shipped for gold score)
  EF           = exp(fsub)            (ACT, bf16)
  chunk scan: At <- (Eblk.T @ At) * EF_s   (PE matmul + DVE j-major bcast mul)

Positions are column-permuted on the host so that each scan step reads a
contiguous 32-column EF slice, and the 8 sentences (groups) are pair-packed
at partition offsets {0,16} inside 32-aligned slots (PE transposes write
[32,128] blocks at legal partition starts).

Host combines the 8192 chunk matrices (f64, tree per sentence + sequential
sentence carry) and computes the gold score from the shipped fsub + m.
"""
import numpy as np
import ml_dtypes
from contextlib import ExitStack

import concourse.bass as bass
import concourse.mybir as mybir
from concourse.tile import TileContext
from concourse.bass_utils import run_bass_kernel_spmd

B, S, H, T = 64, 512, 768, 12
START, STOP, NEG = 10, 11, -10000.0
L = 4                      # chunk length (positions per transfer matrix)
NCORES = 8
P_CORE = B * S // NCORES   # 4096 positions per core
G = 8                      # sentences (groups) per core
KPG = S // L               # 128 chunks per group
NQ = 4                     # quarter chains
CPQ = KPG // NQ            # 32 chunks per group per quarter
NLIVE = 10

FP8NP = ml_dtypes.float8_e4m3fn
BF16 = ml_dtypes.bfloat16

F32 = mybir.dt.float32
BF = mybir.dt.bfloat16
FP8 = mybir.dt.float8e4

# device column <-> original position permutation (per core)
# position q = g*512 + k*L + s ; Q = k//CPQ ; c = k%CPQ
# col = Q*1024 + g*128 + s*32 + c
_cols = np.arange(P_CORE)
_Q = _cols // 1024
_g = (_cols % 1024) // 128
_s = (_cols % 128) // 32
_c = _cols % 32
PERM = (_g * S + (_Q * CPQ + _c) * L + _s)   # PERM[col] = original position


def _build_nc():
    nc = bass.Bass()
    hidT = nc.declare_dram_parameter("hidT", [H, P_CORE], FP8, isOutput=False)
    cf8 = nc.declare_dram_parameter("cf8", [128, 212], FP8, isOutput=False)
    cb16 = nc.declare_dram_parameter("cb16", [128, 576], BF, isOutput=False)
    fm_out = nc.declare_dram_parameter("fm_out", [128, 416], BF, isOutput=True)
    a_out = nc.declare_dram_parameter("a_out", [128, NQ * 320], BF, isOutput=True)

    with ExitStack() as ctx:
        tc = ctx.enter_context(TileContext(nc))
        const_pool = ctx.enter_context(tc.tile_pool(name="const", bufs=1))
        hid_pool = ctx.enter_context(tc.tile_pool(name="hid", bufs=12))
        sb_pool = ctx.enter_context(tc.tile_pool(name="sb", bufs=1))
        at_pool = ctx.enter_context(tc.tile_pool(name="at", bufs=8))
        fp_pool = ctx.enter_context(tc.tile_pool(name="fps", bufs=1, space="PSUM"))
        eft_pool = ctx.enter_context(tc.tile_pool(name="eftp", bufs=1, space="PSUM"))
        ps_pool = ctx.enter_context(tc.tile_pool(name="pss", bufs=4, space="PSUM"))
        warm_pool = ctx.enter_context(tc.tile_pool(name="warm", bufs=1, space="PSUM"))

        # ---- persistent SBUF tiles ----
        cf8_sb = const_pool.tile([128, 212], FP8)
        cb16_sb = const_pool.tile([128, 576], BF)
        efpos = sb_pool.tile([128, 512], BF)       # pos-major EF, pair-packed
        ef_sb = sb_pool.tile([128, 512], BF)       # tag-major EF (scan operand)
        fraw = sb_pool.tile([128, 384], BF)        # feats psum copied to SBUF
        fm_sb = sb_pool.tile([128, 416], BF)       # m (cols 0:32) | fsub (32:416)
        a_sb = sb_pool.tile([128, NQ * 320], BF)   # final chunk matrices

        fp_ps = fp_pool.tile([128, 384], F32)      # feats psum (32 blocks x 12)
        eft_ps = eft_pool.tile([128, 512], BF)     # transposed EF psum

        # ---- pad init: zero the 4-wide pad columns of efpos (cols g*16+12..16)
        nc.gpsimd.memset(
            bass.AP(efpos.tensor, efpos[:, 12:16].offset,
                    [efpos[:, :].ap[0], [16, 32], [1, 4]]),
            0.0,
        )

        # ---- input DMAs, split across the three DMA-capable queues ----
        # half h covers cols h*2048:(h+1)*2048 ; row block hs covers 128 h-rows
        hid = {}
        def hdma(eng, hs, h):
            t = hid_pool.tile([128, 2048], FP8, name=f"hid_{hs}_{h}", tag="hid")
            eng.dma_start(out=t[:, :],
                          in_=hidT[hs * 128:(hs + 1) * 128, h * 2048:(h + 1) * 2048])
            hid[(hs, h)] = t

        # Pool: 5 hid ; SP: 5 hid + cb16 ; ACT: cf8 + 2 hid
        nc.scalar.dma_start(out=cf8_sb[:, :], in_=cf8[:, :])
        hdma(nc.gpsimd, 0, 0)
        hdma(nc.sync, 3, 0)
        hdma(nc.gpsimd, 1, 0)
        hdma(nc.sync, 4, 0)
        hdma(nc.scalar, 5, 0)
        hdma(nc.gpsimd, 2, 0)
        nc.sync.dma_start(out=cb16_sb[:, :], in_=cb16[:, :])
        hdma(nc.sync, 2, 1)
        hdma(nc.gpsimd, 0, 1)
        hdma(nc.sync, 3, 1)
        hdma(nc.gpsimd, 1, 1)
        hdma(nc.sync, 4, 1)
        hdma(nc.scalar, 5, 1)

        ident = cb16_sb[:, 0:128]
        eblk = cb16_sb[:, 128:256]
        e40jm = cb16_sb[:, 256:576]

        # ---- warmups: ramp PE p-state, load ACT exp table early ----
        wp = warm_pool.tile([128, 384], F32)
        scr = const_pool.tile([1, 16], BF)
        nc.scalar.activation(scr[0:1, 0:8], cf8_sb[0:1, 0:8],
                             mybir.ActivationFunctionType.Exp)
        for _ in range(3):
            nc.tensor.matmul(wp[:, 0:212], lhsT=cf8_sb[:, 0:128],
                             rhs=cf8_sb[:, 0:212], start=True, stop=True)

        def feats_quarter(Q):
            h = Q // 2
            for g in range(G):
                blk = fp_ps[:, (Q * 8 + g) * 12:(Q * 8 + g) * 12 + 12]
                for hs in range(6):
                    nc.tensor.matmul(
                        blk,
                        lhsT=hid[(hs, h)][:, (Q % 2) * 1024 + g * 128:
                                          (Q % 2) * 1024 + (g + 1) * 128],
                        rhs=cf8_sb[:, hs * 12:(hs + 1) * 12],
                        start=(hs == 0), stop=False,
                        skip_group_check=True,
                    )
                nc.tensor.matmul(
                    blk, lhsT=cf8_sb[0:1, 72:200], rhs=cf8_sb[0:1, 200:212],
                    start=False, stop=True, skip_group_check=True,
                )
            # copy feats psum -> SBUF bf16 (ACT; gpsimd cannot touch PSUM)
            nc.scalar.activation(
                fraw[:, Q * 96:(Q + 1) * 96], fp_ps[:, Q * 96:(Q + 1) * 96],
                mybir.ActivationFunctionType.Copy,
            )
            fq3 = fraw[:, Q * 96:(Q + 1) * 96].rearrange("p (b j) -> p b j", j=12)
            # m = max over live tags (DVE, bf16 2x)
            nc.vector.reduce_max(
                out=fm_sb[:, Q * 8:(Q + 1) * 8],
                in_=bass.AP(fq3.tensor, fq3.offset,
                            [fq3.ap[0], fq3.ap[1], [1, NLIVE]]),
                axis=mybir.AxisListType.X,
            )
            # fsub = feats - m (Pool, all SBUF)
            msl = fm_sb[:, Q * 8:(Q + 1) * 8]
            m_b = bass.AP(msl.tensor, msl.offset,
                          [msl.ap[0], msl.ap[1], [0, 12]])
            nc.gpsimd.tensor_sub(
                fm_sb[:, 32 + Q * 96:32 + (Q + 1) * 96]
                .rearrange("p (b j) -> p b j", j=12),
                fq3, m_b,
            )
            # EF = exp(fsub) into pair-packed layout (ACT)
            eo = efpos[:, Q * 128:(Q + 1) * 128]
            eo3 = bass.AP(eo.tensor, eo.offset, [eo.ap[0], [16, 8], [1, 12]])
            nc.scalar.activation(
                eo3,
                fm_sb[:, 32 + Q * 96:32 + (Q + 1) * 96]
                .rearrange("p (b j) -> p b j", j=12),
                mybir.ActivationFunctionType.Exp,
            )
            # transpose pairs into tag-major psum
            for v in range(4):
                nc.tensor.transpose(
                    eft_ps[32 * v:32 * v + 32, Q * 128:(Q + 1) * 128],
                    efpos[:, Q * 128 + v * 32:Q * 128 + v * 32 + 32],
                    ident,
                    tile_position=(0, 32 * v),
                )
            # copy psum -> sbuf (DVE, 2x)
            nc.vector.tensor_copy(ef_sb[:, Q * 128:(Q + 1) * 128],
                                  eft_ps[:, Q * 128:(Q + 1) * 128])

        def ef_ap(Q, s):
            base = ef_sb[:, Q * 128 + s * 32:Q * 128 + s * 32 + 32]
            return bass.AP(base.tensor, base.offset,
                           [base.ap[0], [0, NLIVE], base.ap[1]])

        def scan_quarter(Q):
            at = at_pool.tile([128, 320], BF, name=f"at_{Q}_0", tag="at")
            nc.vector.tensor_mul(
                at[:, :].rearrange("p (j c) -> p j c", c=32),
                e40jm.rearrange("p (j c) -> p j c", c=32),
                ef_ap(Q, 0),
            )
            for s in range(1, L):
                ps = ps_pool.tile([128, 320], F32)
                nc.tensor.matmul(ps[:, :], lhsT=eblk, rhs=at[:, :],
                                 start=True, stop=True)
                if s < L - 1:
                    at2 = at_pool.tile([128, 320], BF, name=f"at_{Q}_{s}", tag="at")
                    out_ap = at2[:, :].rearrange("p (j c) -> p j c", c=32)
                else:
                    at2 = None
                    sl = a_sb[:, Q * 320:(Q + 1) * 320]
                    out_ap = sl.rearrange("p (j c) -> p j c", c=32)
                nc.vector.tensor_mul(
                    out_ap,
                    ps[:, :].rearrange("p (j c) -> p j c", c=32),
                    ef_ap(Q, s),
                )
                at = at2

        for Q in range(NQ):
            feats_quarter(Q)
            scan_quarter(Q)
            if Q == 1:
                nc.sync.dma_start(out=a_out[:, 0:640], in_=a_sb[:, 0:640])
        nc.sync.dma_start(out=fm_out[:, :], in_=fm_sb[:, :])
        nc.scalar.dma_start(out=a_out[:, 640:1280], in_=a_sb[:, 640:1280])
    return nc


_NC_CACHE = None


def _get_nc():
    global _NC_CACHE
    if _NC_CACHE is None:
        _NC_CACHE = _build_nc()
    return _NC_CACHE


def _build_consts(W, b, transitions):
    E = np.exp(transitions.astype(np.float64))
    E[START, :] = 0.0
    E[STOP, :] = 0.0
    E[:, STOP] = 0.0
    E = E.astype(np.float32)

    cf8 = np.zeros((128, 212), np.float32)
    # wt: cf8[p, hs*12+t] = W[t, hs*128+p]
    cf8[:, 0:72] = W.T.reshape(6, 128, T).transpose(1, 0, 2).reshape(128, 72)
    cf8[0, 72:200] = 1.0
    cf8[0, 200:212] = b
    cf8 = cf8.astype(FP8NP)

    cb16 = np.zeros((128, 576), np.float32)
    cb16[:, 0:128] = np.eye(128)
    # eblk: eblk[32v+off+j, 32v+off+i] = E[i, j]  (live 10x10)
    for v in range(4):
        for off in (0, 16):
            o = 32 * v + off
            cb16[o:o + NLIVE, 128 + o:128 + o + NLIVE] = E[:NLIVE, :NLIVE].T
    # e40jm: e40jm[32v+off+i, j*32+c] = E[i, j]
    blk = np.zeros((32, 320), np.float32)
    for off in (0, 16):
        for i in range(NLIVE):
            for j in range(NLIVE):
                blk[off + i, j * 32:(j + 1) * 32] = E[i, j]
    for v in range(4):
        cb16[32 * v:32 * v + 32, 256:576] = blk
    cb16 = cb16.astype(BF16)
    return cf8, cb16


def _run_device(hidden, W, b, transitions, trace=False, tmpdir=None):
    cf8, cb16 = _build_consts(W, b, transitions)
    flat = hidden.reshape(B * S, H)
    in_maps = []
    for core in range(NCORES):
        blk = flat[core * P_CORE:(core + 1) * P_CORE]        # [4096, 768]
        hT = np.ascontiguousarray(blk[PERM].T).astype(FP8NP)  # [768, 4096]
        in_maps.append({"hidT": hT, "cf8": cf8, "cb16": cb16})
    return run_bass_kernel_spmd(
        _get_nc(), in_maps, list(range(NCORES)), trace=trace, tmpdir=tmpdir)


def _logsumexp(x, axis):
    mx = np.max(x, axis=axis)
    mx_safe = np.where(np.isfinite(mx), mx, 0.0)
    out = mx + np.log(np.sum(np.exp(x - np.expand_dims(mx_safe, axis)), axis=axis))
    return np.where(np.isfinite(mx), out, -np.inf)


def _host_combine(results, transitions, tags):
    trans = transitions.astype(np.float64)
    err = np.errstate(invalid="ignore", divide="ignore", over="ignore")
    err.__enter__()

    # unpack fm_out: m and fsub in device (permuted) order -> original order
    feats = np.zeros((NCORES, P_CORE, T), np.float64)
    m_all = np.zeros((NCORES, P_CORE), np.float64)
    for core, r in enumerate(results):
        fm = np.asarray(r["fm_out"]).astype(np.float64)   # [128, 416]
        # device col layout: col = Q*1024 + g*128 + s*32 + c ; tile t=Q*8+g,
        # partition p = s*32+c ; m = fm[:, t], fsub = fm[:, 32+12t+j]
        m_dev = fm[:, 0:32]            # [p, t]
        fs_dev = fm[:, 32:416].reshape(128, 32, 12)   # [p, t, j]
        # device column index for (t, p): col = (t//8)*1024 + (t%8)*128 + p
        tt = np.arange(32)
        pp = np.arange(128)
        colidx = (tt[None, :] // 8) * 1024 + (tt[None, :] % 8) * 128 + pp[:, None]
        q = PERM[colidx]               # [p, t] original position
        m_all[core, q] = m_dev
        feats[core, q, :] = fs_dev + m_dev[:, :, None]

    feats = feats.reshape(B, S, T)     # [B, S, T] (= fsub + m, exact device feats)
    m_all = m_all.reshape(B, S)

    # unpack chunk matrices: A[b, k][i, j] (live 10x10), log + scale
    logA = np.zeros((B, KPG, NLIVE, NLIVE), np.float64)
    for core, r in enumerate(results):
        a = np.asarray(r["a_out"]).astype(np.float64)     # [128, 1280]
        a4 = a.reshape(128, NQ, NLIVE, 32)                 # [p, Q, j, c]
        for g in range(G):
            rows = 32 * (g // 2) + 16 * (g % 2)
            blkm = a4[rows:rows + NLIVE]                   # [i, Q, j, c]
            logA[core * G + g] = np.log(blkm).transpose(1, 3, 0, 2).reshape(
                KPG, NLIVE, NLIVE)
    scale = m_all.reshape(B, KPG, L).sum(axis=2)           # [B, KPG]
    logA = logA + scale[:, :, None, None]

    # first global chunk: explicit recurrence from init (full 12-state)
    v0 = np.full(T, NEG, np.float64)
    v0[START] = 0.0
    for s in range(L):
        v0 = _logsumexp(trans[None, :, :] + v0[None, None, :], axis=2)[0] \
            + feats[0, s]
    # replace chunk (0,0) with identity in the tree
    ident = np.full((NLIVE, NLIVE), -np.inf)
    np.fill_diagonal(ident, 0.0)
    logA[0, 0] = ident

    # tree-combine the 128 chunk mats of each sentence -> one mat per sentence
    mats = logA.reshape(B * KPG, NLIVE, NLIVE)
    n = B * KPG
    while n > B:
        A2 = mats[0::2]        # earlier chunk
        B2 = mats[1::2]        # later chunk
        x = B2[:, :, :, None] + A2[:, None, :, :]          # [n/2, i, j, k]
        mats = _logsumexp(x, axis=2)
        n //= 2

    # sequential carry across sentences
    last = np.zeros((B, T), np.float64)
    v = v0.copy()
    for b in range(B):
        if b == 0:
            vl = _logsumexp(mats[0] + v[None, :NLIVE], axis=1)
        else:
            vl = _logsumexp(mats[b] + v[None, :NLIVE], axis=1)
        v = np.concatenate([vl, [-np.inf, -np.inf]])
        last[b] = v
        if b + 1 < B:
            # start of next sentence: nothing special (carryover CRF)
            pass
    forward_score = _logsumexp(last + trans[STOP][None, :], axis=1)
    err.__exit__(None, None, None)

    tags = np.asarray(tags)
    tags_ext = np.concatenate(
        [np.full((B, 1), START, dtype=tags.dtype), tags], axis=1)
    prev, nxt = tags_ext[:, :-1], tags_ext[:, 1:]
    trans_sc = trans[nxt, prev].sum(axis=1)
    emit_sc = np.take_along_axis(
        feats, nxt[..., None].astype(np.int64), axis=2)[..., 0].sum(axis=1)
    gold = trans_sc + emit_sc + trans[STOP, tags_ext[:, -1]]
    gold_cum = np.cumsum(gold)
    out = np.sum(forward_score - gold_cum)
    return np.array([out], dtype=np.float32)


def kernel(hidden, W, b, transitions, tags, _trace=False, _tmpdir=None):
    hidden = np.asarray(hidden, dtype=np.float32)
    W = np.asarray(W, dtype=np.float32)
    b = np.asarray(b, dtype=np.float32)
    transitions = np.asarray(transitions, dtype=np.float32)
    res = _run_device(hidden, W, b, transitions, trace=_trace, tmpdir=_tmpdir)
    out = _host_combine(res.results, transitions, tags)
    if _trace:
        return out, res
    return out
